# revision 2
# baseline (speedup 1.0000x reference)
"""kNN hypergraph kernel for Trainium2 (8 NeuronCores, Bass/Tile).

Problem: x [16, 256, 768] f32, k=16.
  flat = x.reshape(4096, 768)
  d2[i,j] = |flat_i - flat_j|^2 ; idx = 16 nearest (incl self)
  hypergraph[i, idx[i,:]] = 1 ; out[b,s,t] = sum_b2 hg[b*256+s, b2*256+t]
Output: [16, 256, 256] f32 (per-row histogram of neighbor_index % 256).

Strategy (row-sharded across 8 cores, 512 rows each):
  - Rank rows by s[i,j] = 2*<x_i, x_j> - |x_j|^2  (= sq_i - d2[i,j]; the
    per-row constant sq_i does not change per-row ranking). The 16 NN are
    the 16 LARGEST s per row.
  - Matmul in fp16 hi/lo split (3 cross terms, ~fp32-accurate products at
    full PE speed): s = 2x_hi@x_hi' + 2x_hi@x_lo' + 2x_lo@x_hi' - sq.
    The -sq hi/lo rows ride as two K=1 matmuls (ones stationary).
  - Top-16 per row: per 512-column block, DVE max8 + match_replace + max8
    gives the block top-16 (pipelines with PE); a tiny combine pass over
    the 8x16 union yields sigma = 16th largest of the row.
  - Neighbor mask (s >= sigma) fused with the first histogram fold, then
    binary-tree adds fold the 16 blocks of 256 (sum over batch axis).
"""

import os

import numpy as np

B, S, D = 16, 256, 768
N = B * S            # 4096 points
NCORES = 8
M = N // NCORES      # 512 rows per core
KT = 6               # K tiles of 128 (768 features); -sq rides as K=1 row
KR = D + 1           # 769 rows in the rhs DRAM tensors (row 768 = -sq)
NT = N // 512        # 8 moving tiles of 512 columns
RT = M // 128        # 4 row-tiles of 128 per core
NEG = -3.0e38        # sentinel: far below any real s value (~|s| < 1e5)

_cache = {}


def _build():
    import concourse.mybir as mybir
    import concourse.tile as tile
    from concourse import bacc

    f32 = mybir.dt.float32
    f16 = mybir.dt.float16
    bf16 = mybir.dt.bfloat16

    nc = bacc.Bacc("TRN2", target_bir_lowering=False, debug=False,
                   num_devices=NCORES)

    rh_d = nc.dram_tensor("rhs_hi", [KR, N], f16, kind="ExternalInput")
    rl_d = nc.dram_tensor("rhs_lo", [KR, N], f16, kind="ExternalInput")
    lh_d = nc.dram_tensor("lhs_hi", [D, M], f16, kind="ExternalInput")
    ll_d = nc.dram_tensor("lhs_lo", [D, M], f16, kind="ExternalInput")
    out_d = nc.dram_tensor("out", [M, S], f32, kind="ExternalOutput")

    with tile.TileContext(nc) as tc:
        with (
            tc.tile_pool(name="weights", bufs=1) as wpool,
            tc.tile_pool(name="s", bufs=2) as spool,
            tc.tile_pool(name="mask", bufs=2) as mpool,
            tc.tile_pool(name="m16", bufs=2) as m16pool,
            tc.tile_pool(name="blk", bufs=3) as blkpool,
            tc.tile_pool(name="m8", bufs=4) as m8pool,
            tc.tile_pool(name="outp", bufs=4) as opool,
            tc.tile_pool(name="psum", bufs=8, space="PSUM") as psum,
        ):
            rh_sb, rl_sb, lh_sb, ll_sb = [], [], [], []
            for ki in range(KT):
                t = wpool.tile([128, N], f16, tag=f"rh{ki}", name=f"rh{ki}")
                nc.sync.dma_start(out=t, in_=rh_d[ki * 128:(ki + 1) * 128, :])
                rh_sb.append(t)
                t = wpool.tile([128, N], f16, tag=f"rl{ki}", name=f"rl{ki}")
                nc.sync.dma_start(out=t, in_=rl_d[ki * 128:(ki + 1) * 128, :])
                rl_sb.append(t)
                t = wpool.tile([128, M], f16, tag=f"lh{ki}", name=f"lh{ki}")
                nc.sync.dma_start(out=t, in_=lh_d[ki * 128:(ki + 1) * 128, :])
                lh_sb.append(t)
                t = wpool.tile([128, M], f16, tag=f"ll{ki}", name=f"ll{ki}")
                nc.sync.dma_start(out=t, in_=ll_d[ki * 128:(ki + 1) * 128, :])
                ll_sb.append(t)
            sq_h = wpool.tile([1, N], f16, tag="sq_h", name="sq_h")
            nc.sync.dma_start(out=sq_h, in_=rh_d[D:D + 1, :])
            sq_l = wpool.tile([1, N], f16, tag="sq_l", name="sq_l")
            nc.sync.dma_start(out=sq_l, in_=rl_d[D:D + 1, :])
            ones = wpool.tile([1, 128], f16, tag="ones", name="ones")
            nc.vector.memset(ones, 1.0)

            for rt in range(RT):
                rsl = slice(rt * 128, (rt + 1) * 128)
                s_sb = spool.tile([128, N], f32, tag="s", name="s_sb")
                m16 = m16pool.tile([128, 8 * 16], f32, tag="m16", name="m16")
                ps = [psum.tile([128, 512], f32, tag="ps", name=f"ps{n}")
                      for n in range(NT)]

                def mm(n, ki, pi):
                    lw, rm = [
                        (lh_sb[ki][:, rsl], rh_sb[ki]),
                        (lh_sb[ki][:, rsl], rl_sb[ki]),
                        (ll_sb[ki][:, rsl], rh_sb[ki]),
                    ][pi]
                    nc.tensor.matmul(
                        ps[n][:, :], lw, rm[:, n * 512:(n + 1) * 512],
                        start=(ki == 0 and pi == 0), stop=False)

                def mm_sq(n):
                    # two K=1 matmuls add the -sq row (hi then lo)
                    nsl = slice(n * 512, (n + 1) * 512)
                    nc.tensor.matmul(ps[n][:, :], ones, sq_h[:, nsl],
                                     start=False, stop=False)
                    nc.tensor.matmul(ps[n][:, :], ones, sq_l[:, nsl],
                                     start=False, stop=True)

                def drain_block(n):
                    # PSUM -> SBUF, then per-block top-16 into m16
                    nsl = slice(n * 512, (n + 1) * 512)
                    nc.scalar.copy(out=s_sb[:, nsl], in_=ps[n][:, :])
                    a8 = m16[:, n * 16:n * 16 + 8]
                    b8 = m16[:, n * 16 + 8:n * 16 + 16]
                    scr = blkpool.tile([128, 512], f32, tag="scr", name="scr")
                    nc.vector.max(out=a8, in_=s_sb[:, nsl])
                    nc.vector.match_replace(out=scr, in_to_replace=a8,
                                            in_values=s_sb[:, nsl],
                                            imm_value=NEG)
                    nc.vector.max(out=b8, in_=scr)

                if rt == 0:
                    # first row-tile: K-outer so PE starts as DMA tiles land
                    for ki in range(KT):
                        for pi in range(3):
                            for n in range(NT):
                                mm(n, ki, pi)
                    for n in range(NT):
                        mm_sq(n)
                        drain_block(n)
                else:
                    # weights resident: N-outer so drains pipeline with PE
                    for n in range(NT):
                        for ki in range(KT):
                            for pi in range(3):
                                mm(n, ki, pi)
                        mm_sq(n)
                        drain_block(n)

                # sigma = 16th largest of the union of block top-16s
                c8 = m8pool.tile([128, 8], f32, tag="c8", name="c8")
                m16x = m16pool.tile([128, 8 * 16], f32, tag="m16x", name="m16x")
                d8 = m8pool.tile([128, 8], f32, tag="d8", name="d8")
                nc.vector.max(out=c8, in_=m16)
                nc.vector.match_replace(out=m16x, in_to_replace=c8,
                                        in_values=m16, imm_value=NEG)
                nc.vector.max(out=d8, in_=m16x)
                sigma = d8[:, 7:8]

                # neighbor mask (s >= sigma), fused with first 2048-fold
                H = N // 2
                mask = mpool.tile([128, H], bf16, tag="mask", name="mask")
                nc.vector.tensor_scalar(mask, s_sb[:, :H], sigma, None,
                                        op0=mybir.AluOpType.is_ge)
                nc.vector.scalar_tensor_tensor(
                    out=mask, in0=s_sb[:, H:], scalar=sigma, in1=mask,
                    op0=mybir.AluOpType.is_ge, op1=mybir.AluOpType.add)
                w = H // 2
                while w > S:
                    nc.vector.tensor_add(mask[:, :w], mask[:, :w],
                                         mask[:, w:2 * w])
                    w //= 2
                o = opool.tile([128, S], f32, tag="o", name="o")
                nc.vector.tensor_add(o, mask[:, :S], mask[:, S:2 * S])
                nc.sync.dma_start(out=out_d[rsl, :], in_=o)

    nc.compile()
    return nc


def _prep_inputs(x):
    flat = np.asarray(x, dtype=np.float32).reshape(N, D)
    sq = (flat.astype(np.float64) ** 2).sum(1).astype(np.float32)

    hi = flat.astype(np.float16)
    lo = (flat - hi.astype(np.float32)).astype(np.float16)
    hi2 = (2.0 * flat).astype(np.float16)
    lo2 = (2.0 * flat - hi2.astype(np.float32)).astype(np.float16)
    nsq_h = (-sq).astype(np.float16)
    nsq_l = (-sq - nsq_h.astype(np.float32)).astype(np.float16)

    rhs_hi = np.empty((KR, N), dtype=np.float16)
    rhs_hi[:D] = hi.T
    rhs_hi[D] = nsq_h
    rhs_lo = np.empty((KR, N), dtype=np.float16)
    rhs_lo[:D] = lo.T
    rhs_lo[D] = nsq_l
    lhs_hi = np.ascontiguousarray(hi2.T)   # [768, 4096]
    lhs_lo = np.ascontiguousarray(lo2.T)
    return rhs_hi, rhs_lo, lhs_hi, lhs_lo


def kernel(x, k):
    assert int(k) == 16
    rhs_hi, rhs_lo, lhs_hi, lhs_lo = _prep_inputs(x)

    if "nc" not in _cache:
        _cache["nc"] = _build()
    nc = _cache["nc"]

    in_maps = [
        {"rhs_hi": rhs_hi, "rhs_lo": rhs_lo,
         "lhs_hi": np.ascontiguousarray(lhs_hi[:, c * M:(c + 1) * M]),
         "lhs_lo": np.ascontiguousarray(lhs_lo[:, c * M:(c + 1) * M])}
        for c in range(NCORES)
    ]

    from concourse.bass_utils import run_bass_kernel_spmd
    trace = bool(os.environ.get("KNN_TRACE"))
    if trace:
        try:
            from antenv.axon_hooks import get_axon_ntff_profile_hook  # noqa
        except ImportError:
            trace = False
    res = run_bass_kernel_spmd(nc, in_maps, core_ids=list(range(NCORES)),
                               trace=trace)
    if trace:
        _cache["res"] = res
    if trace and res.exec_time_ns is not None:
        print(f"HW exec time: {res.exec_time_ns} ns")
        _cache["exec_time_ns"] = res.exec_time_ns

    out = np.concatenate([r["out"] for r in res.results], axis=0)
    return out.reshape(B, S, S)

